# revision 22
# baseline (speedup 1.0000x reference)
"""Trainium2 Bass kernel for nn_CrossAttention_44693429682227 (v2).

Math (reference):
    q = (x @ Wq.T) / E**0.25, reshaped (b, t, H, E)
    scores = q @ keys.T over a shared bank of N=50000 (key, scalar-value) pairs
    attn = softmax(scores, axis=-1)
    out = mean_h(attn @ values) + curiosity  -> (b, t, 1)

out_row = (sum_n e_n * v_n) / (sum_n e_n) with e_n = exp(s_n - C); the global
shift C cancels in the ratio and keeps e_n inside fp8e5 range. C is computed
at runtime from the actual global max score (cheap host matmul) and shipped
to the cores as a DMA'd constant, so any input realization is safe.

Distribution: key bank sharded 8 ways (NBLK=50 blocks of 128 keys per core);
every core computes all 512 query rows x 8 heads against its shard and emits
partial num/den sums; host merges in f64.

Per-core pipeline, per head, over 25 block-PAIRS (2 PSUM banks each):
  PE:   scoresT[pair] = keysT.T @ qT            (f32r, 2 x 512 cols)
  split exp across two engines to beat the ACT-only exp roofline:
   - ACT pairs (13/25): eT = exp(s - C) -> fp8e5 (spline exp + RNE convert).
   - DVE pairs (12/25): 8-bit Schraudolph: u8 = sat_round(s*A8 + B8) (DVE
     f32->u8 converts with RNE + [0,255] saturation); u8 bit pattern IS
     fp8e5(~e^(s-C)) (+-6%, zero-mean).
  Then num_hi/num_lo/den accumulate via ONE DoubleRow fp8 matmul per pair
  (vb8 [128,2,16] e4m3 x eT [128,2,512] e5m2, ~216ns for 256 keys): all 25
  DRs are emitted as one uninterrupted burst per head AFTER the score phase,
  avoiding PE fp32<->fp8 mode-switch stalls. The next head's q projection is
  software-pipelined one chunk per pair-slot; dummy warm-up matmuls ramp the
  PE clock while the first DMAs land.

kernel.py is self-contained: shapes/sharding hardcoded, no sibling imports.
"""

import os
import sys
from contextlib import ExitStack

import numpy as np

if "/opt/trn_rl_repo" not in sys.path:
    sys.path.insert(0, "/opt/trn_rl_repo")

import ml_dtypes

# Problem shapes (hardcoded per contract)
B, T = 4, 128
BT = B * T            # 512 query (b,t) rows
HIN = 1024
H, E = 8, 128
N = 50000
NCORES = 8

# Sharding / tiling
NBLK = 50             # 128-key blocks per core
NPAIR = NBLK // 2     # 25 pairs (2 PSUM banks / pair)
KC = NBLK * 128       # 6400 keys per core
NPAD = KC * NCORES    # 51200 padded bank size
KCH = HIN // 128      # 8 contraction chunks for the projection
NKCHUNK = 5           # keysT DMA'd in 5 chunks of 5 pairs

# exp split and numerics
NDVE = int(os.environ.get("KNDVE", "12"))         # pairs per head on DVE
SIGMA8 = 0.05596213                               # zero-mean 8-bit Schraudolph
A8 = 4.0 * 1.4426950408889634                     # 4*log2(e)
E5_LN_MAX = 10.96                                 # ln(57344)
C_MARGIN = 0.56                                   # s-units of Inf headroom
TRACE = bool(int(os.environ.get("KTRACE", "0")))

LAST_RESULTS = None   # BassKernelResults of the most recent run (for test.py)

_cache = {}


def _dve_pairs():
    return set(int((i + 0.5) * NPAIR / NDVE) for i in range(NDVE))


def _install_ntff_hook():
    """Register the axon NTFF profile hook that this image's antenv lacks."""
    import types

    if "antenv.axon_hooks" in sys.modules:
        return
    try:
        from trn_agent_boot.trn_boot import _ntff_profile_via_ctypes

        hook = _ntff_profile_via_ctypes("/opt/axon/libaxon_pjrt.so")
    except Exception:
        hook = None
    mod = types.ModuleType("antenv.axon_hooks")
    mod.get_axon_ntff_profile_hook = lambda: hook
    sys.modules["antenv.axon_hooks"] = mod

    from concourse import bass_utils as bu

    orig_upload = bu.upload_artifacts

    def safe_upload(tmpdir):
        try:
            return orig_upload(tmpdir)
        except Exception as e:
            return f"upload-skipped ({type(e).__name__})"

    bu.upload_artifacts = safe_upload


def _build():
    import concourse.bass as bass
    import concourse.tile as tile
    from concourse import bacc, mybir

    f32 = mybir.dt.float32
    f32r = mybir.dt.float32r
    bf16 = mybir.dt.bfloat16
    u8dt = mybir.dt.uint8
    u16 = mybir.dt.uint16
    fp8e5 = mybir.dt.float8e5
    fp8e4 = mybir.dt.float8e4
    DR = mybir.MatmulPerfMode.DoubleRow
    Exp = mybir.ActivationFunctionType.Exp

    dve_set = _dve_pairs()

    nc = bacc.Bacc(trn_type="TRN2", target_bir_lowering=False, debug=False)

    # Host pre-arranges xt/wqt so every DMA is one contiguous run per
    # partition: xt[p, k, bt] = x[bt, 128k+p]; wqt[h, p, k, e] = Wq.T[128k+p, 128h+e]
    proj_dt = bf16 if os.environ.get("KPROJ", "f32r") == "bf16" else f32r
    xt_d = nc.dram_tensor("xt", [128, KCH * BT], proj_dt, kind="ExternalInput")
    wqt_d = nc.dram_tensor("wqt", [H, 128, KCH * E], proj_dt, kind="ExternalInput")
    keyst_d = nc.dram_tensor("keyst", [E, KC], f32r, kind="ExternalInput")
    vb8_d = nc.dram_tensor("vb8", [128, NPAIR * 2 * 16], fp8e4, kind="ExternalInput")
    # runtime exp-shift constants: col 0 = -C (ACT bias), col 1 = B8 (DVE trick)
    cvec_d = nc.dram_tensor("cvec", [128, 2], f32, kind="ExternalInput")
    nd_d = nc.dram_tensor("nd_out", [3, H * BT], f32, kind="ExternalOutput")

    with tile.TileContext(nc) as tc, ExitStack() as ctx:
        singles = ctx.enter_context(tc.tile_pool(name="singles", bufs=1))
        epool = ctx.enter_context(tc.tile_pool(name="epool", bufs=int(os.environ.get("KEBUFS", "16"))))
        upool = ctx.enter_context(tc.tile_pool(name="upool", bufs=int(os.environ.get("KUBUFS", "15"))))
        ps_s = ctx.enter_context(tc.tile_pool(name="ps_s", bufs=int(os.environ.get("KSBUFS", "3")), space="PSUM"))
        ps_q = ctx.enter_context(tc.tile_pool(name="ps_q", bufs=int(os.environ.get("KQBUFS", "1")), space="PSUM"))
        ps_nd = ctx.enter_context(tc.tile_pool(name="ps_nd", bufs=int(os.environ.get("KNDBUFS", "1")), space="PSUM"))

        # ---- persistent SBUF loads, critical-path-first ----
        def load(pool, name, shape, dtype, src):
            t = pool.tile(shape, dtype, name=name, tag=name)
            nc.sync.dma_start(out=t, in_=src)
            return t

        wq_h = [None] * H
        xt_k = [None] * KCH
        keyst_c = [None] * NKCHUNK
        KPC = KC // NKCHUNK  # keys per chunk (1280)

        def load_wq(h):
            wq_h[h] = load(
                singles, f"wq{h}", [128, KCH, E], proj_dt,
                wqt_d.ap()[h].rearrange("p (k e) -> p k e", e=E),
            )

        def load_wq0():
            wq_h[0] = load(
                singles, "wq0", [128, KCH, E], proj_dt,
                wqt_d.ap()[0].rearrange("p (k e) -> p k e", e=E),
            )
            for k in range(KCH):
                xt_k[k] = load(
                    singles, f"xt{k}", [128, BT], proj_dt,
                    xt_d.ap()[:, BT * k:BT * (k + 1)],
                )

        def load_kc(i):
            keyst_c[i] = load(
                singles, f"keyst{i}", [128, KPC], f32r,
                keyst_d.ap()[:, KPC * i:KPC * (i + 1)],
            )

        load_wq0()
        load_kc(0)
        vb8_sb = load(
            singles, "vb8", [128, NPAIR, 2, 16], fp8e4,
            vb8_d.ap().rearrange("p (a b c) -> p a b c", b=2, c=16),
        )
        load_kc(1)
        load_wq(1)
        load_kc(2)
        load_wq(2)
        load_kc(3)
        load_wq(3)
        load_kc(4)
        for h in range(4, H):
            load_wq(h)

        cvec_sb = load(singles, "cvec", [128, 2], f32, cvec_d.ap())
        bias_ap = cvec_sb[:, 0:1]
        b8_ap = cvec_sb[:, 1:2]

        qt_sb = singles.tile([128, H, BT], f32r)
        out_sb = singles.tile([3, H, BT], f32)

        def key_block(blk):  # lhsT AP for 128-key block blk
            ch, off = divmod(blk * 128, KPC)
            return keyst_c[ch][:, off:off + 128]

        # PE pstate warm-up: dummy matmuls on garbage SBUF while the first
        # DMAs land, so real matmuls start at full clock
        nwarm = int(os.environ.get("KWARM", "10"))
        warm_src = singles.tile([128, BT], f32)
        nc.gpsimd.memset(warm_src, 0.0)
        warm_v = warm_src.bitcast(f32r)
        for w in range(nwarm):
            warm_ps = ps_s.tile([128, 2, BT], f32, tag="s", name=f"warm{w}")
            nc.tensor.matmul(
                warm_ps[:, 0, :], lhsT=warm_v[:, 0:128],
                rhs=warm_v, start=True, stop=True,
            )

        q_ps_cur = [None]

        def proj_chunk(h, k):
            if k == 0:
                q_ps_cur[0] = ps_q.tile([128, BT], f32, tag="q", name=f"q_ps{h}")
            nc.tensor.matmul(
                q_ps_cur[0], lhsT=wq_h[h][:, k, :], rhs=xt_k[k],
                start=(k == 0), stop=(k == KCH - 1),
            )
            if k == KCH - 1:
                if os.environ.get("KCOPYACT", "1") == "1":
                    nc.scalar.copy(qt_sb[:, h, :], q_ps_cur[0])
                else:
                    nc.vector.tensor_copy(qt_sb[:, h, :], q_ps_cur[0])

        for k in range(KCH):
            proj_chunk(0, k)
        for h in range(H):
            nd_ps = ps_nd.tile([16, BT], f32, tag="nd", name=f"nd_ps{h}")
            rhs_es = []
            # phase A: scores stream through exp engines; eT tiles buffered
            for p in range(NPAIR):
                s_ps = ps_s.tile([128, 2, BT], f32, tag="s", name=f"s_{h}_{p}")
                for j in range(2):
                    nc.tensor.matmul(
                        s_ps[:, j, :], lhsT=key_block(2 * p + j),
                        rhs=qt_sb[:, h, :], start=True, stop=True,
                    )
                if p in dve_set:
                    eT8 = upool.tile([128, 2, BT], u8dt, tag="u", name=f"u_{h}_{p}")
                    if os.environ.get("KDVEBF", "0") == "1":
                        s_in = s_ps.bitcast(u16).rearrange(
                            "p a (n two) -> p a n two", two=2)[:, :, :, 1].bitcast(bf16)
                    else:
                        s_in = s_ps[:, :, :]
                    nc.vector.tensor_scalar(
                        eT8, s_in, A8, b8_ap,
                        mybir.AluOpType.mult, mybir.AluOpType.add,
                    )
                    rhs_es.append(eT8.bitcast(fp8e5))
                else:
                    eT = epool.tile([128, 2, BT], fp8e5, tag="e", name=f"e_{h}_{p}")
                    nc.scalar.activation(eT, s_ps, Exp, bias=bias_ap)
                    rhs_es.append(eT[:, :, :])
                if p < KCH and h + 1 < H:
                    proj_chunk(h + 1, p)
            # phase B: one uninterrupted fp8-DR burst (no PE mode switches)
            for p in range(NPAIR):
                nc.tensor.matmul(
                    nd_ps, lhsT=vb8_sb[:, p, :, :], rhs=rhs_es[p],
                    perf_mode=DR, start=(p == 0), stop=(p == NPAIR - 1),
                    skip_group_check=True,
                )
            if os.environ.get("KCOPYACT", "1") == "1":
                nc.scalar.copy(out_sb[:, h, :], nd_ps[0:3, :])
            else:
                nc.vector.tensor_copy(out_sb[:, h, :], nd_ps[0:3, :])

        nc.sync.dma_start(out=nd_d.ap(), in_=out_sb.rearrange("p h b -> p (h b)"))

    nc.compile()
    return nc


def _prep_inputs(x, Wq, keys, values):
    f8e4 = ml_dtypes.float8_e4m3fn
    proj_np = ml_dtypes.bfloat16 if os.environ.get("KPROJ", "f32r") == "bf16" else np.float32
    # xt[p, k, bt] = x[bt, 128k+p]  (one contiguous run per partition)
    xT = np.ascontiguousarray(
        np.asarray(x, dtype=np.float32).reshape(BT, KCH, 128).transpose(2, 1, 0)
    ).reshape(128, KCH * BT).astype(proj_np)
    # wqt[h, p, k, e] = Wq.T[128k+p, 128h+e], with 1/E**0.25 folded in
    wq_s = np.asarray(Wq, dtype=np.float32) * np.float32(E ** -0.25)  # [oc, hin]
    wqT = np.ascontiguousarray(
        wq_s.reshape(H, E, KCH, 128).transpose(0, 3, 2, 1)  # [h, p, k, e]
    ).reshape(H, 128, KCH * E).astype(proj_np)

    keys_pad = np.zeros((NPAD, E), dtype=np.float32)
    keys_pad[:N] = np.asarray(keys, dtype=np.float32)
    keysT = np.ascontiguousarray(keys_pad.T)  # [E, NPAD]

    # exp shift C from the actual global max score (cheap host matmul) so
    # exp(s - C) never reaches fp8e5 Inf regardless of input realization
    q = (np.asarray(x, np.float32).reshape(BT, HIN) @ wq_s.T).reshape(BT, H, E)
    max_s = max(float((q[:, h, :] @ keys_pad.T).max()) for h in range(H))
    c_shift = max_s - E5_LN_MAX + C_MARGIN
    b8 = np.float32(4.0 * (15.0 - SIGMA8) - A8 * c_shift)
    cvec = np.zeros((128, 2), dtype=np.float32)
    cvec[:, 0] = np.float32(-c_shift)
    cvec[:, 1] = b8

    v_pad = np.zeros(NPAD, dtype=np.float32)
    v_pad[:N] = np.asarray(values, dtype=np.float32)
    mask = np.zeros(NPAD, dtype=np.float32)
    mask[:N] = 1.0
    v_hi = np.clip(v_pad, -240, 240).astype(f8e4).astype(np.float32)
    v_lo = (v_pad - v_hi).astype(f8e4).astype(np.float32)

    # [NPAD] -> [NCORES, NBLK, 128] -> per-core block-major
    def shard(a):
        return a.reshape(NCORES, NBLK, 128)

    v_hi_s, v_lo_s, mask_s = shard(v_hi), shard(v_lo), shard(mask)

    # vb8[core][p(=key in block), pair, j, col16]: cols 0..2 = v_hi, v_lo, mask
    vb8 = np.zeros((NCORES, 128, NPAIR, 2, 16), dtype=np.float32)
    for col, src in enumerate((v_hi_s, v_lo_s, mask_s)):
        vb8[:, :, :, :, col] = src.reshape(
            NCORES, NPAIR, 2, 128).transpose(0, 3, 1, 2)
    vb8 = vb8.astype(f8e4)

    in_maps = []
    for c in range(NCORES):
        in_maps.append(
            {
                "xt": xT,
                "wqt": wqT,
                "keyst": np.ascontiguousarray(keysT[:, c * KC:(c + 1) * KC]),
                "vb8": np.ascontiguousarray(vb8[c].reshape(128, NPAIR * 2 * 16)),
                "cvec": cvec,
            }
        )
    return in_maps


def kernel(x, curiosity_score, Wq, keys, values):
    global LAST_RESULTS
    if TRACE:
        _install_ntff_hook()
    from concourse.bass_utils import run_bass_kernel_spmd

    key = (NDVE,)
    if key not in _cache:
        _cache[key] = _build()
    nc = _cache[key]

    in_maps = _prep_inputs(x, Wq, keys, values)

    res = run_bass_kernel_spmd(
        nc, in_maps, core_ids=list(range(NCORES)), trace=TRACE
    )
    LAST_RESULTS = res

    nd = np.stack(
        [np.asarray(res.results[c]["nd_out"], dtype=np.float64) for c in range(NCORES)]
    ).reshape(NCORES, 3, H, BT)
    num = (nd[:, 0] + nd[:, 1]).sum(axis=0)  # [H, BT]
    den = nd[:, 2].sum(axis=0)               # [H, BT]
    out = (num / den).mean(axis=0) + np.asarray(
        curiosity_score, dtype=np.float64
    ).reshape(BT)
    return out.astype(np.float32).reshape(B, T, 1)


# revision 23
# speedup vs baseline: 1.0861x; 1.0861x over previous
"""Trainium2 Bass kernel for nn_CrossAttention_44693429682227 (v2).

Math (reference):
    q = (x @ Wq.T) / E**0.25, reshaped (b, t, H, E)
    scores = q @ keys.T over a shared bank of N=50000 (key, scalar-value) pairs
    attn = softmax(scores, axis=-1)
    out = mean_h(attn @ values) + curiosity  -> (b, t, 1)

out_row = (sum_n e_n * v_n) / (sum_n e_n) with e_n = exp(s_n - C); the global
shift C cancels in the ratio and keeps e_n inside fp8e5 range. C is computed
at runtime from the actual global max score (cheap host matmul) and shipped
to the cores as a DMA'd constant, so any input realization is safe.

Distribution: key bank sharded 8 ways (NBLK=50 blocks of 128 keys per core);
every core computes all 512 query rows x 8 heads against its shard and emits
partial num/den sums; host merges in f64.

Per-core pipeline, per head, over 25 block-PAIRS (2 PSUM banks each):
  PE:   scoresT[pair] = keysT.T @ qT            (f32r, 2 x 512 cols)
  split exp across two engines to beat the ACT-only exp roofline:
   - ACT pairs (13/25): eT = exp(s - C) -> fp8e5 (spline exp + RNE convert).
   - DVE pairs (12/25): 8-bit Schraudolph: u8 = sat_round(s*A8 + B8) (DVE
     f32->u8 converts with RNE + [0,255] saturation); u8 bit pattern IS
     fp8e5(~e^(s-C)) (+-6%, zero-mean).
  Then num_hi/num_lo/den accumulate via ONE DoubleRow fp8 matmul per pair
  (vb8 [128,2,16] e4m3 x eT [128,2,512] e5m2, ~216ns for 256 keys): all 25
  DRs are emitted as one uninterrupted burst per head AFTER the score phase,
  avoiding PE fp32<->fp8 mode-switch stalls. The next head's q projection is
  software-pipelined one chunk per pair-slot; dummy warm-up matmuls ramp the
  PE clock while the first DMAs land.

kernel.py is self-contained: shapes/sharding hardcoded, no sibling imports.
"""

import os
import sys
from contextlib import ExitStack

import numpy as np

if "/opt/trn_rl_repo" not in sys.path:
    sys.path.insert(0, "/opt/trn_rl_repo")

import ml_dtypes

# Problem shapes (hardcoded per contract)
B, T = 4, 128
BT = B * T            # 512 query (b,t) rows
HIN = 1024
H, E = 8, 128
N = 50000
NCORES = 8

# Sharding / tiling
NBLK = 50             # 128-key blocks per core
NPAIR = NBLK // 2     # 25 pairs (2 PSUM banks / pair)
KC = NBLK * 128       # 6400 keys per core
NPAD = KC * NCORES    # 51200 padded bank size
KCH = HIN // 128      # 8 contraction chunks for the projection
NKCHUNK = 5           # keysT DMA'd in 5 chunks of 5 pairs

# exp split and numerics
NDVE = int(os.environ.get("KNDVE", "12"))         # pairs per head on DVE
SIGMA8 = 0.05596213                               # zero-mean 8-bit Schraudolph
A8 = 4.0 * 1.4426950408889634                     # 4*log2(e)
E5_LN_MAX = 10.96                                 # ln(57344)
C_MARGIN = 0.56                                   # s-units of Inf headroom
TRACE = bool(int(os.environ.get("KTRACE", "0")))

LAST_RESULTS = None   # BassKernelResults of the most recent run (for test.py)

_cache = {}


def _dve_pairs():
    return set(int((i + 0.5) * NPAIR / NDVE) for i in range(NDVE))


def _install_ntff_hook():
    """Register the axon NTFF profile hook that this image's antenv lacks."""
    import types

    if "antenv.axon_hooks" in sys.modules:
        return
    try:
        from trn_agent_boot.trn_boot import _ntff_profile_via_ctypes

        hook = _ntff_profile_via_ctypes("/opt/axon/libaxon_pjrt.so")
    except Exception:
        hook = None
    mod = types.ModuleType("antenv.axon_hooks")
    mod.get_axon_ntff_profile_hook = lambda: hook
    sys.modules["antenv.axon_hooks"] = mod

    from concourse import bass_utils as bu

    orig_upload = bu.upload_artifacts

    def safe_upload(tmpdir):
        try:
            return orig_upload(tmpdir)
        except Exception as e:
            return f"upload-skipped ({type(e).__name__})"

    bu.upload_artifacts = safe_upload


def _build():
    import concourse.bass as bass
    import concourse.tile as tile
    from concourse import bacc, mybir

    f32 = mybir.dt.float32
    f32r = mybir.dt.float32r
    bf16 = mybir.dt.bfloat16
    u8dt = mybir.dt.uint8
    u16 = mybir.dt.uint16
    fp8e5 = mybir.dt.float8e5
    fp8e4 = mybir.dt.float8e4
    DR = mybir.MatmulPerfMode.DoubleRow
    Exp = mybir.ActivationFunctionType.Exp

    dve_set = _dve_pairs()

    nc = bacc.Bacc(trn_type="TRN2", target_bir_lowering=False, debug=False)

    # Host pre-arranges xt/wqt so every DMA is one contiguous run per
    # partition: xt[p, k, bt] = x[bt, 128k+p]; wqt[h, p, k, e] = Wq.T[128k+p, 128h+e]
    proj_dt = bf16 if os.environ.get("KPROJ", "f32r") == "bf16" else f32r
    xt_d = nc.dram_tensor("xt", [128, KCH * BT], proj_dt, kind="ExternalInput")
    wqt_d = nc.dram_tensor("wqt", [H, 128, KCH * E], proj_dt, kind="ExternalInput")
    key_dt = bf16 if os.environ.get("KKEYBF", "0") == "1" else f32r
    keyst_d = nc.dram_tensor("keyst", [E, KC], key_dt, kind="ExternalInput")
    vb8_d = nc.dram_tensor("vb8", [128, NPAIR * 2 * 16], fp8e4, kind="ExternalInput")
    # runtime exp-shift constants: col 0 = -C (ACT bias), col 1 = B8 (DVE trick)
    cvec_d = nc.dram_tensor("cvec", [128, 2], f32, kind="ExternalInput")
    nd_d = nc.dram_tensor("nd_out", [3, H * BT], f32, kind="ExternalOutput")

    with tile.TileContext(nc) as tc, ExitStack() as ctx:
        singles = ctx.enter_context(tc.tile_pool(name="singles", bufs=1))
        epool = ctx.enter_context(tc.tile_pool(name="epool", bufs=int(os.environ.get("KEBUFS", "16"))))
        upool = ctx.enter_context(tc.tile_pool(name="upool", bufs=int(os.environ.get("KUBUFS", "15"))))
        ps_s = ctx.enter_context(tc.tile_pool(name="ps_s", bufs=int(os.environ.get("KSBUFS", "3")), space="PSUM"))
        ps_q = ctx.enter_context(tc.tile_pool(name="ps_q", bufs=int(os.environ.get("KQBUFS", "1")), space="PSUM"))
        ps_nd = ctx.enter_context(tc.tile_pool(name="ps_nd", bufs=int(os.environ.get("KNDBUFS", "1")), space="PSUM"))

        # ---- persistent SBUF loads, critical-path-first ----
        def load(pool, name, shape, dtype, src):
            t = pool.tile(shape, dtype, name=name, tag=name)
            nc.sync.dma_start(out=t, in_=src)
            return t

        wq_h = [None] * H
        xt_k = [None] * KCH
        keyst_c = [None] * NKCHUNK
        KPC = KC // NKCHUNK  # keys per chunk (1280)

        def load_wq(h):
            wq_h[h] = load(
                singles, f"wq{h}", [128, KCH, E], proj_dt,
                wqt_d.ap()[h].rearrange("p (k e) -> p k e", e=E),
            )

        def load_wq0():
            wq_h[0] = load(
                singles, "wq0", [128, KCH, E], proj_dt,
                wqt_d.ap()[0].rearrange("p (k e) -> p k e", e=E),
            )
            for k in range(KCH):
                xt_k[k] = load(
                    singles, f"xt{k}", [128, BT], proj_dt,
                    xt_d.ap()[:, BT * k:BT * (k + 1)],
                )

        def load_kc(i):
            keyst_c[i] = load(
                singles, f"keyst{i}", [128, KPC], key_dt,
                keyst_d.ap()[:, KPC * i:KPC * (i + 1)],
            )

        load_wq0()
        load_kc(0)
        vb8_sb = load(
            singles, "vb8", [128, NPAIR, 2, 16], fp8e4,
            vb8_d.ap().rearrange("p (a b c) -> p a b c", b=2, c=16),
        )
        load_kc(1)
        load_wq(1)
        load_kc(2)
        load_wq(2)
        load_kc(3)
        load_wq(3)
        load_kc(4)
        for h in range(4, H):
            load_wq(h)

        cvec_sb = load(singles, "cvec", [128, 2], f32, cvec_d.ap())
        bias_ap = cvec_sb[:, 0:1]
        b8_ap = cvec_sb[:, 1:2]

        qt_dt = bf16 if os.environ.get("KQTBF", "0") == "1" else f32r
        qt_sb = singles.tile([128, H, BT], qt_dt)
        out_sb = singles.tile([3, H, BT], f32)

        def key_block(blk):  # lhsT AP for 128-key block blk
            ch, off = divmod(blk * 128, KPC)
            return keyst_c[ch][:, off:off + 128]

        # PE pstate warm-up: dummy matmuls on garbage SBUF while the first
        # DMAs land, so real matmuls start at full clock
        nwarm = int(os.environ.get("KWARM", "10"))
        warm_src = singles.tile([128, BT], f32)
        nc.gpsimd.memset(warm_src, 0.0)
        warm_v = warm_src.bitcast(f32r)
        for w in range(nwarm):
            warm_ps = ps_s.tile([128, 2, BT], f32, tag="s", name=f"warm{w}")
            nc.tensor.matmul(
                warm_ps[:, 0, :], lhsT=warm_v[:, 0:128],
                rhs=warm_v, start=True, stop=True,
            )

        q_ps_cur = [None]

        def proj_chunk(h, k):
            if k == 0:
                q_ps_cur[0] = ps_q.tile([128, BT], f32, tag="q", name=f"q_ps{h}")
            nc.tensor.matmul(
                q_ps_cur[0], lhsT=wq_h[h][:, k, :], rhs=xt_k[k],
                start=(k == 0), stop=(k == KCH - 1),
            )
            if k == KCH - 1:
                if os.environ.get("KCOPYACT", "1") == "1":
                    nc.scalar.copy(qt_sb[:, h, :], q_ps_cur[0])
                else:
                    nc.vector.tensor_copy(qt_sb[:, h, :], q_ps_cur[0])

        for k in range(KCH):
            proj_chunk(0, k)
        for h in range(H):
            nd_ps = ps_nd.tile([16, BT], f32, tag="nd", name=f"nd_ps{h}")
            rhs_es = []
            # phase A: scores stream through exp engines; eT tiles buffered
            for p in range(NPAIR):
                s_ps = ps_s.tile([128, 2, BT], f32, tag="s", name=f"s_{h}_{p}")
                for j in range(2):
                    nc.tensor.matmul(
                        s_ps[:, j, :], lhsT=key_block(2 * p + j),
                        rhs=qt_sb[:, h, :], start=True, stop=True,
                    )
                if p in dve_set:
                    eT8 = upool.tile([128, 2, BT], u8dt, tag="u", name=f"u_{h}_{p}")
                    if os.environ.get("KDVEBF", "0") == "1":
                        s_in = s_ps.bitcast(u16).rearrange(
                            "p a (n two) -> p a n two", two=2)[:, :, :, 1].bitcast(bf16)
                    else:
                        s_in = s_ps[:, :, :]
                    nc.vector.tensor_scalar(
                        eT8, s_in, A8, b8_ap,
                        mybir.AluOpType.mult, mybir.AluOpType.add,
                    )
                    rhs_es.append(eT8.bitcast(fp8e5))
                else:
                    eT = epool.tile([128, 2, BT], fp8e5, tag="e", name=f"e_{h}_{p}")
                    nc.scalar.activation(eT, s_ps, Exp, bias=bias_ap)
                    rhs_es.append(eT[:, :, :])
                if p < KCH and h + 1 < H:
                    proj_chunk(h + 1, p)
            # phase B: one uninterrupted fp8-DR burst (no PE mode switches)
            for p in range(NPAIR):
                nc.tensor.matmul(
                    nd_ps, lhsT=vb8_sb[:, p, :, :], rhs=rhs_es[p],
                    perf_mode=DR, start=(p == 0), stop=(p == NPAIR - 1),
                    skip_group_check=True,
                )
            if os.environ.get("KCOPYACT", "1") == "1":
                nc.scalar.copy(out_sb[:, h, :], nd_ps[0:3, :])
            else:
                nc.vector.tensor_copy(out_sb[:, h, :], nd_ps[0:3, :])

        nc.sync.dma_start(out=nd_d.ap(), in_=out_sb.rearrange("p h b -> p (h b)"))

    nc.compile()
    return nc


def _prep_inputs(x, Wq, keys, values):
    f8e4 = ml_dtypes.float8_e4m3fn
    proj_np = ml_dtypes.bfloat16 if os.environ.get("KPROJ", "f32r") == "bf16" else np.float32
    # xt[p, k, bt] = x[bt, 128k+p]  (one contiguous run per partition)
    xT = np.ascontiguousarray(
        np.asarray(x, dtype=np.float32).reshape(BT, KCH, 128).transpose(2, 1, 0)
    ).reshape(128, KCH * BT).astype(proj_np)
    # wqt[h, p, k, e] = Wq.T[128k+p, 128h+e], with 1/E**0.25 folded in
    wq_s = np.asarray(Wq, dtype=np.float32) * np.float32(E ** -0.25)  # [oc, hin]
    wqT = np.ascontiguousarray(
        wq_s.reshape(H, E, KCH, 128).transpose(0, 3, 2, 1)  # [h, p, k, e]
    ).reshape(H, 128, KCH * E).astype(proj_np)

    keys_pad = np.zeros((NPAD, E), dtype=np.float32)
    keys_pad[:N] = np.asarray(keys, dtype=np.float32)
    key_np = ml_dtypes.bfloat16 if os.environ.get("KKEYBF", "0") == "1" else np.float32
    keysT = np.ascontiguousarray(keys_pad.T).astype(key_np)  # [E, NPAD]

    # exp shift C from the actual global max score (cheap host matmul) so
    # exp(s - C) never reaches fp8e5 Inf regardless of input realization
    q = (np.asarray(x, np.float32).reshape(BT, HIN) @ wq_s.T).reshape(BT, H, E)
    max_s = max(float((q[:, h, :] @ keys_pad.T).max()) for h in range(H))
    c_shift = max_s - E5_LN_MAX + C_MARGIN
    b8 = np.float32(4.0 * (15.0 - SIGMA8) - A8 * c_shift)
    cvec = np.zeros((128, 2), dtype=np.float32)
    cvec[:, 0] = np.float32(-c_shift)
    cvec[:, 1] = b8

    v_pad = np.zeros(NPAD, dtype=np.float32)
    v_pad[:N] = np.asarray(values, dtype=np.float32)
    mask = np.zeros(NPAD, dtype=np.float32)
    mask[:N] = 1.0
    v_hi = np.clip(v_pad, -240, 240).astype(f8e4).astype(np.float32)
    v_lo = (v_pad - v_hi).astype(f8e4).astype(np.float32)

    # [NPAD] -> [NCORES, NBLK, 128] -> per-core block-major
    def shard(a):
        return a.reshape(NCORES, NBLK, 128)

    v_hi_s, v_lo_s, mask_s = shard(v_hi), shard(v_lo), shard(mask)

    # vb8[core][p(=key in block), pair, j, col16]: cols 0..2 = v_hi, v_lo, mask
    vb8 = np.zeros((NCORES, 128, NPAIR, 2, 16), dtype=np.float32)
    for col, src in enumerate((v_hi_s, v_lo_s, mask_s)):
        vb8[:, :, :, :, col] = src.reshape(
            NCORES, NPAIR, 2, 128).transpose(0, 3, 1, 2)
    vb8 = vb8.astype(f8e4)

    in_maps = []
    for c in range(NCORES):
        in_maps.append(
            {
                "xt": xT,
                "wqt": wqT,
                "keyst": np.ascontiguousarray(keysT[:, c * KC:(c + 1) * KC]),
                "vb8": np.ascontiguousarray(vb8[c].reshape(128, NPAIR * 2 * 16)),
                "cvec": cvec,
            }
        )
    return in_maps


def kernel(x, curiosity_score, Wq, keys, values):
    global LAST_RESULTS
    if TRACE:
        _install_ntff_hook()
    from concourse.bass_utils import run_bass_kernel_spmd

    key = (NDVE,)
    if key not in _cache:
        _cache[key] = _build()
    nc = _cache[key]

    in_maps = _prep_inputs(x, Wq, keys, values)

    res = run_bass_kernel_spmd(
        nc, in_maps, core_ids=list(range(NCORES)), trace=TRACE
    )
    LAST_RESULTS = res

    nd = np.stack(
        [np.asarray(res.results[c]["nd_out"], dtype=np.float64) for c in range(NCORES)]
    ).reshape(NCORES, 3, H, BT)
    num = (nd[:, 0] + nd[:, 1]).sum(axis=0)  # [H, BT]
    den = nd[:, 2].sum(axis=0)               # [H, BT]
    out = (num / den).mean(axis=0) + np.asarray(
        curiosity_score, dtype=np.float64
    ).reshape(BT)
    return out.astype(np.float32).reshape(B, T, 1)


# revision 24
# speedup vs baseline: 1.1153x; 1.0269x over previous
"""Trainium2 Bass kernel for nn_CrossAttention_44693429682227 (v2).

Math (reference):
    q = (x @ Wq.T) / E**0.25, reshaped (b, t, H, E)
    scores = q @ keys.T over a shared bank of N=50000 (key, scalar-value) pairs
    attn = softmax(scores, axis=-1)
    out = mean_h(attn @ values) + curiosity  -> (b, t, 1)

out_row = (sum_n e_n * v_n) / (sum_n e_n) with e_n = exp(s_n - C); the global
shift C cancels in the ratio and keeps e_n inside fp8e5 range. C is computed
at runtime from the actual global max score (cheap host matmul) and shipped
to the cores as a DMA'd constant, so any input realization is safe.

Distribution: key bank sharded 8 ways (NBLK=50 blocks of 128 keys per core);
every core computes all 512 query rows x 8 heads against its shard and emits
partial num/den sums; host merges in f64.

Per-core pipeline, per head, over 25 block-PAIRS (2 PSUM banks each):
  PE:   scoresT[pair] = keysT.T @ qT            (f32r, 2 x 512 cols)
  split exp across two engines to beat the ACT-only exp roofline:
   - ACT pairs (13/25): eT = exp(s - C) -> fp8e5 (spline exp + RNE convert).
   - DVE pairs (12/25): 8-bit Schraudolph: u8 = sat_round(s*A8 + B8) (DVE
     f32->u8 converts with RNE + [0,255] saturation); u8 bit pattern IS
     fp8e5(~e^(s-C)) (+-6%, zero-mean).
  Then num_hi/num_lo/den accumulate via ONE DoubleRow fp8 matmul per pair
  (vb8 [128,2,16] e4m3 x eT [128,2,512] e5m2, ~216ns for 256 keys): all 25
  DRs are emitted as one uninterrupted burst per head AFTER the score phase,
  avoiding PE fp32<->fp8 mode-switch stalls. The next head's q projection is
  software-pipelined one chunk per pair-slot; dummy warm-up matmuls ramp the
  PE clock while the first DMAs land.

kernel.py is self-contained: shapes/sharding hardcoded, no sibling imports.
"""

import os
import sys
from contextlib import ExitStack

import numpy as np

if "/opt/trn_rl_repo" not in sys.path:
    sys.path.insert(0, "/opt/trn_rl_repo")

import ml_dtypes

# Problem shapes (hardcoded per contract)
B, T = 4, 128
BT = B * T            # 512 query (b,t) rows
HIN = 1024
H, E = 8, 128
N = 50000
NCORES = 8

# Sharding / tiling
NBLK = 50             # 128-key blocks per core
NPAIR = NBLK // 2     # 25 pairs (2 PSUM banks / pair)
KC = NBLK * 128       # 6400 keys per core
NPAD = KC * NCORES    # 51200 padded bank size
KCH = HIN // 128      # 8 contraction chunks for the projection
NKCHUNK = 5           # keysT DMA'd in 5 chunks of 5 pairs

# exp split and numerics
NDVE = int(os.environ.get("KNDVE", "12"))         # pairs per head on DVE
SIGMA8 = 0.05596213                               # zero-mean 8-bit Schraudolph
A8 = 4.0 * 1.4426950408889634                     # 4*log2(e)
E5_LN_MAX = 10.96                                 # ln(57344)
C_MARGIN = 0.56                                   # s-units of Inf headroom
TRACE = bool(int(os.environ.get("KTRACE", "0")))

LAST_RESULTS = None   # BassKernelResults of the most recent run (for test.py)

_cache = {}


def _dve_pairs():
    return set(int((i + 0.5) * NPAIR / NDVE) for i in range(NDVE))


def _install_ntff_hook():
    """Register the axon NTFF profile hook that this image's antenv lacks."""
    import types

    if "antenv.axon_hooks" in sys.modules:
        return
    try:
        from trn_agent_boot.trn_boot import _ntff_profile_via_ctypes

        hook = _ntff_profile_via_ctypes("/opt/axon/libaxon_pjrt.so")
    except Exception:
        hook = None
    mod = types.ModuleType("antenv.axon_hooks")
    mod.get_axon_ntff_profile_hook = lambda: hook
    sys.modules["antenv.axon_hooks"] = mod

    from concourse import bass_utils as bu

    orig_upload = bu.upload_artifacts

    def safe_upload(tmpdir):
        try:
            return orig_upload(tmpdir)
        except Exception as e:
            return f"upload-skipped ({type(e).__name__})"

    bu.upload_artifacts = safe_upload


def _build():
    import concourse.bass as bass
    import concourse.tile as tile
    from concourse import bacc, mybir

    f32 = mybir.dt.float32
    f32r = mybir.dt.float32r
    bf16 = mybir.dt.bfloat16
    u8dt = mybir.dt.uint8
    u16 = mybir.dt.uint16
    fp8e5 = mybir.dt.float8e5
    fp8e4 = mybir.dt.float8e4
    DR = mybir.MatmulPerfMode.DoubleRow
    Exp = mybir.ActivationFunctionType.Exp

    dve_set = _dve_pairs()

    nc = bacc.Bacc(trn_type="TRN2", target_bir_lowering=False, debug=False)

    # Host pre-arranges xt/wqt so every DMA is one contiguous run per
    # partition: xt[p, k, bt] = x[bt, 128k+p]; wqt[h, p, k, e] = Wq.T[128k+p, 128h+e]
    proj_dt = bf16 if os.environ.get("KPROJ", "bf16") == "bf16" else f32r
    xt_d = nc.dram_tensor("xt", [128, KCH * BT], proj_dt, kind="ExternalInput")
    wqt_d = nc.dram_tensor("wqt", [H, 128, KCH * E], proj_dt, kind="ExternalInput")
    key_dt = bf16 if os.environ.get("KKEYBF", "1") == "1" else f32r
    keyst_d = nc.dram_tensor("keyst", [E, KC], key_dt, kind="ExternalInput")
    vb8_d = nc.dram_tensor("vb8", [128, NPAIR * 2 * 16], fp8e4, kind="ExternalInput")
    # runtime exp-shift constants: col 0 = -C (ACT bias), col 1 = B8 (DVE trick)
    cvec_d = nc.dram_tensor("cvec", [128, 2], f32, kind="ExternalInput")
    nd_d = nc.dram_tensor("nd_out", [3, H * BT], f32, kind="ExternalOutput")

    with tile.TileContext(nc) as tc, ExitStack() as ctx:
        singles = ctx.enter_context(tc.tile_pool(name="singles", bufs=1))
        epool = ctx.enter_context(tc.tile_pool(name="epool", bufs=int(os.environ.get("KEBUFS", "16"))))
        upool = ctx.enter_context(tc.tile_pool(name="upool", bufs=int(os.environ.get("KUBUFS", "15"))))
        ps_s = ctx.enter_context(tc.tile_pool(name="ps_s", bufs=int(os.environ.get("KSBUFS", "3")), space="PSUM"))
        ps_q = ctx.enter_context(tc.tile_pool(name="ps_q", bufs=int(os.environ.get("KQBUFS", "1")), space="PSUM"))
        ps_nd = ctx.enter_context(tc.tile_pool(name="ps_nd", bufs=int(os.environ.get("KNDBUFS", "1")), space="PSUM"))

        # ---- persistent SBUF loads, critical-path-first ----
        def load(pool, name, shape, dtype, src):
            t = pool.tile(shape, dtype, name=name, tag=name)
            nc.sync.dma_start(out=t, in_=src)
            return t

        wq_h = [None] * H
        xt_k = [None] * KCH
        keyst_c = [None] * NKCHUNK
        KPC = KC // NKCHUNK  # keys per chunk (1280)

        def load_wq(h):
            wq_h[h] = load(
                singles, f"wq{h}", [128, KCH, E], proj_dt,
                wqt_d.ap()[h].rearrange("p (k e) -> p k e", e=E),
            )

        def load_wq0():
            wq_h[0] = load(
                singles, "wq0", [128, KCH, E], proj_dt,
                wqt_d.ap()[0].rearrange("p (k e) -> p k e", e=E),
            )
            for k in range(KCH):
                xt_k[k] = load(
                    singles, f"xt{k}", [128, BT], proj_dt,
                    xt_d.ap()[:, BT * k:BT * (k + 1)],
                )

        def load_kc(i):
            keyst_c[i] = load(
                singles, f"keyst{i}", [128, KPC], key_dt,
                keyst_d.ap()[:, KPC * i:KPC * (i + 1)],
            )

        load_wq0()
        load_kc(0)
        vb8_sb = load(
            singles, "vb8", [128, NPAIR, 2, 16], fp8e4,
            vb8_d.ap().rearrange("p (a b c) -> p a b c", b=2, c=16),
        )
        load_kc(1)
        load_wq(1)
        load_kc(2)
        load_wq(2)
        load_kc(3)
        load_wq(3)
        load_kc(4)
        for h in range(4, H):
            load_wq(h)

        cvec_sb = load(singles, "cvec", [128, 2], f32, cvec_d.ap())
        bias_ap = cvec_sb[:, 0:1]
        b8_ap = cvec_sb[:, 1:2]

        qt_dt = bf16 if os.environ.get("KQTBF", "1") == "1" else f32r
        qt_sb = singles.tile([128, H, BT], qt_dt)
        out_sb = singles.tile([3, H, BT], f32)

        def key_block(blk):  # lhsT AP for 128-key block blk
            ch, off = divmod(blk * 128, KPC)
            return keyst_c[ch][:, off:off + 128]

        # PE pstate warm-up: dummy matmuls on garbage SBUF while the first
        # DMAs land, so real matmuls start at full clock
        nwarm = int(os.environ.get("KWARM", "10"))
        warm_src = singles.tile([128, BT], f32)
        nc.gpsimd.memset(warm_src, 0.0)
        warm_v = warm_src.bitcast(bf16)
        for w in range(nwarm):
            warm_ps = ps_s.tile([128, 2, BT], f32, tag="s", name=f"warm{w}")
            nc.tensor.matmul(
                warm_ps[:, 0, :], lhsT=warm_v[:, 0:128],
                rhs=warm_v[:, 0:BT], start=True, stop=True,
            )

        q_ps_cur = [None]

        def proj_chunk(h, k):
            if k == 0:
                q_ps_cur[0] = ps_q.tile([128, BT], f32, tag="q", name=f"q_ps{h}")
            nc.tensor.matmul(
                q_ps_cur[0], lhsT=wq_h[h][:, k, :], rhs=xt_k[k],
                start=(k == 0), stop=(k == KCH - 1),
            )
            if k == KCH - 1:
                if os.environ.get("KCOPYACT", "1") == "1":
                    nc.scalar.copy(qt_sb[:, h, :], q_ps_cur[0])
                else:
                    nc.vector.tensor_copy(qt_sb[:, h, :], q_ps_cur[0])

        for k in range(KCH):
            proj_chunk(0, k)
        for h in range(H):
            nd_ps = ps_nd.tile([16, BT], f32, tag="nd", name=f"nd_ps{h}")
            rhs_es = []
            # phase A: scores stream through exp engines; eT tiles buffered
            for p in range(NPAIR):
                s_ps = ps_s.tile([128, 2, BT], f32, tag="s", name=f"s_{h}_{p}")
                for j in range(2):
                    nc.tensor.matmul(
                        s_ps[:, j, :], lhsT=key_block(2 * p + j),
                        rhs=qt_sb[:, h, :], start=True, stop=True,
                    )
                if p in dve_set:
                    eT8 = upool.tile([128, 2, BT], u8dt, tag="u", name=f"u_{h}_{p}")
                    if os.environ.get("KDVEBF", "0") == "1":
                        s_in = s_ps.bitcast(u16).rearrange(
                            "p a (n two) -> p a n two", two=2)[:, :, :, 1].bitcast(bf16)
                    else:
                        s_in = s_ps[:, :, :]
                    nc.vector.tensor_scalar(
                        eT8, s_in, A8, b8_ap,
                        mybir.AluOpType.mult, mybir.AluOpType.add,
                    )
                    rhs_es.append(eT8.bitcast(fp8e5))
                else:
                    eT = epool.tile([128, 2, BT], fp8e5, tag="e", name=f"e_{h}_{p}")
                    nc.scalar.activation(eT, s_ps, Exp, bias=bias_ap)
                    rhs_es.append(eT[:, :, :])
                if p < KCH and h + 1 < H:
                    proj_chunk(h + 1, p)
            # phase B: one uninterrupted fp8-DR burst (no PE mode switches)
            for p in range(NPAIR):
                nc.tensor.matmul(
                    nd_ps, lhsT=vb8_sb[:, p, :, :], rhs=rhs_es[p],
                    perf_mode=DR, start=(p == 0), stop=(p == NPAIR - 1),
                    skip_group_check=True,
                )
            if os.environ.get("KCOPYACT", "1") == "1":
                nc.scalar.copy(out_sb[:, h, :], nd_ps[0:3, :])
            else:
                nc.vector.tensor_copy(out_sb[:, h, :], nd_ps[0:3, :])

        nc.sync.dma_start(out=nd_d.ap(), in_=out_sb.rearrange("p h b -> p (h b)"))

    nc.compile()
    return nc


def _prep_inputs(x, Wq, keys, values):
    f8e4 = ml_dtypes.float8_e4m3fn
    proj_np = ml_dtypes.bfloat16 if os.environ.get("KPROJ", "bf16") == "bf16" else np.float32
    # xt[p, k, bt] = x[bt, 128k+p]  (one contiguous run per partition)
    xT = np.ascontiguousarray(
        np.asarray(x, dtype=np.float32).reshape(BT, KCH, 128).transpose(2, 1, 0)
    ).reshape(128, KCH * BT).astype(proj_np)
    # wqt[h, p, k, e] = Wq.T[128k+p, 128h+e], with 1/E**0.25 folded in
    wq_s = np.asarray(Wq, dtype=np.float32) * np.float32(E ** -0.25)  # [oc, hin]
    wqT = np.ascontiguousarray(
        wq_s.reshape(H, E, KCH, 128).transpose(0, 3, 2, 1)  # [h, p, k, e]
    ).reshape(H, 128, KCH * E).astype(proj_np)

    keys_pad = np.zeros((NPAD, E), dtype=np.float32)
    keys_pad[:N] = np.asarray(keys, dtype=np.float32)
    key_np = ml_dtypes.bfloat16 if os.environ.get("KKEYBF", "1") == "1" else np.float32
    keysT = np.ascontiguousarray(keys_pad.T).astype(key_np)  # [E, NPAD]

    # exp shift C from the actual global max score (cheap host matmul) so
    # exp(s - C) never reaches fp8e5 Inf regardless of input realization
    q = (np.asarray(x, np.float32).reshape(BT, HIN) @ wq_s.T).reshape(BT, H, E)
    max_s = max(float((q[:, h, :] @ keys_pad.T).max()) for h in range(H))
    c_shift = max_s - E5_LN_MAX + C_MARGIN
    b8 = np.float32(4.0 * (15.0 - SIGMA8) - A8 * c_shift)
    cvec = np.zeros((128, 2), dtype=np.float32)
    cvec[:, 0] = np.float32(-c_shift)
    cvec[:, 1] = b8

    v_pad = np.zeros(NPAD, dtype=np.float32)
    v_pad[:N] = np.asarray(values, dtype=np.float32)
    mask = np.zeros(NPAD, dtype=np.float32)
    mask[:N] = 1.0
    v_hi = np.clip(v_pad, -240, 240).astype(f8e4).astype(np.float32)
    v_lo = (v_pad - v_hi).astype(f8e4).astype(np.float32)

    # [NPAD] -> [NCORES, NBLK, 128] -> per-core block-major
    def shard(a):
        return a.reshape(NCORES, NBLK, 128)

    v_hi_s, v_lo_s, mask_s = shard(v_hi), shard(v_lo), shard(mask)

    # vb8[core][p(=key in block), pair, j, col16]: cols 0..2 = v_hi, v_lo, mask
    vb8 = np.zeros((NCORES, 128, NPAIR, 2, 16), dtype=np.float32)
    for col, src in enumerate((v_hi_s, v_lo_s, mask_s)):
        vb8[:, :, :, :, col] = src.reshape(
            NCORES, NPAIR, 2, 128).transpose(0, 3, 1, 2)
    vb8 = vb8.astype(f8e4)

    in_maps = []
    for c in range(NCORES):
        in_maps.append(
            {
                "xt": xT,
                "wqt": wqT,
                "keyst": np.ascontiguousarray(keysT[:, c * KC:(c + 1) * KC]),
                "vb8": np.ascontiguousarray(vb8[c].reshape(128, NPAIR * 2 * 16)),
                "cvec": cvec,
            }
        )
    return in_maps


def kernel(x, curiosity_score, Wq, keys, values):
    global LAST_RESULTS
    if TRACE:
        _install_ntff_hook()
    from concourse.bass_utils import run_bass_kernel_spmd

    key = (NDVE,)
    if key not in _cache:
        _cache[key] = _build()
    nc = _cache[key]

    in_maps = _prep_inputs(x, Wq, keys, values)

    res = run_bass_kernel_spmd(
        nc, in_maps, core_ids=list(range(NCORES)), trace=TRACE
    )
    LAST_RESULTS = res

    nd = np.stack(
        [np.asarray(res.results[c]["nd_out"], dtype=np.float64) for c in range(NCORES)]
    ).reshape(NCORES, 3, H, BT)
    num = (nd[:, 0] + nd[:, 1]).sum(axis=0)  # [H, BT]
    den = nd[:, 2].sum(axis=0)               # [H, BT]
    out = (num / den).mean(axis=0) + np.asarray(
        curiosity_score, dtype=np.float64
    ).reshape(BT)
    return out.astype(np.float32).reshape(B, T, 1)
